# revision 1
# baseline (speedup 1.0000x reference)
"""Multi-head attention block (B=2, N=2048, C=1024, H=16, D=64) on 8 TRN2 cores.

Sharding: core c -> batch b = c // 4, head-group hg = c % 4 (4 heads per core).
Each core:
  qkvT = W_hg @ x_b^T           (fp32r matmuls, xT prepared on host)
  S^T  = kT^T q-chunks          (K=64, two heads row-packed per PE pass)
  P^T  = exp(S^T)               (ScalarE, no max-subtraction: scores ~ N(0,1))
  O^T|Z = [V|1]^T @ P^T         (accumulated over key tiles in PSUM)
  O^T /= Z                      (reciprocal + K=1 broadcast matmul + DVE mul)
  y_partial = O^T^T @ Wp^T      (pair-stacked K=128 matmuls, written to DRAM)
Host sums the 4 head-group partials per batch and adds bias.
"""

import numpy as np

import concourse.bass as bass
import concourse.tile as tile
from concourse import bacc, mybir

F32 = mybir.dt.float32
F32R = mybir.dt.float32r
EXP = mybir.ActivationFunctionType.Exp

B, S, C = 2, 2048, 1024
H, D = 16, 64
HPC = 4            # heads per core
NCT = C // 128     # 8 contraction tiles
MT = S // 128      # 16 key/seq tiles
NCH = S // 512     # 4 query chunks
# fp32r is simulated as exact fp32 in CoreSim; on HW it is the fast 4-byte
# matmul path (1 cyc/row at N>=256). Set to F32 as a (4x slower) fallback.
MM_DT = F32R


def build_bass(loop_n=None):
    nc = bacc.Bacc("TRN2", target_bir_lowering=False)

    xt_d = nc.dram_tensor("xt", [C, S], F32R, kind="ExternalInput")
    wqk_d = nc.dram_tensor("wqk", [C, 512], F32R, kind="ExternalInput")
    wv_d = nc.dram_tensor("wv", [C, 256], F32R, kind="ExternalInput")
    wp_d = nc.dram_tensor("wp", [256, C], F32R, kind="ExternalInput")
    y_d = nc.dram_tensor("y", [S, C], F32, kind="ExternalOutput")

    def mm(out, lhsT, rhs, start, stop):
        nc.tensor.matmul(out, lhsT, rhs, start=start, stop=stop)

    import contextlib

    with tile.TileContext(nc) as tc:
        with tc.tile_pool(name="persist", bufs=1) as persist:
            loop_ctx = tc.For_i(0, loop_n, 1) if loop_n else contextlib.nullcontext()
            # qkT f-tiles: 0=q_h0|q_h1, 1=q_h2|q_h3, 2=k_h0|k_h1, 3=k_h2|k_h3
            qk_sb = persist.tile([128, 4 * S], MM_DT, tag="qk")
            # V augmented per key tile: [v_h0|1|v_h1|1|v_h2|1|v_h3|1] = 260 cols
            vaug = persist.tile([128, MT * 260], MM_DT, tag="vaug")
            wp_sb = persist.tile([64, HPC * C], MM_DT, tag="wp")
            ones_sb = persist.tile([128, 64], F32, tag="ones")
            onorm = [
                persist.tile([64, S], MM_DT, tag=f"onorm{h}", name=f"onorm{h}")
                for h in range(HPC)
            ]

            nc.vector.memset(ones_sb, 1.0)
            for h in range(HPC):
                nc.sync.dma_start(
                    out=wp_sb[:, h * C : (h + 1) * C],
                    in_=wp_d[h * 64 : (h + 1) * 64, :],
                )

            # ---------------- phase A: QKV projections ----------------
            with loop_ctx:
              with (
                  tc.tile_pool(name="ph_a", bufs=1) as ph_a,
                  tc.tile_pool(name="ps_qk", bufs=4, space="PSUM") as ps_qk,
                  tc.tile_pool(name="ps_v", bufs=4, space="PSUM") as ps_v,
              ):
                  xt_sb = ph_a.tile([128, NCT * S], MM_DT, tag="xt")
                  vones = ph_a.tile([128, 260], F32, tag="vones")
                  nc.vector.memset(vones, 1.0)
                  for st in range(MT):
                      nc.vector.tensor_copy(
                          vaug[:, st * 260 : (st + 1) * 260], vones
                      )
                  wqk_sb = ph_a.tile([128, NCT * 512], MM_DT, tag="wqk")
                  wv_sb = ph_a.tile([128, NCT * 256], MM_DT, tag="wv")
                  for ct in range(NCT):
                      nc.sync.dma_start(
                          out=xt_sb[:, ct * S : (ct + 1) * S],
                          in_=xt_d[ct * 128 : (ct + 1) * 128, :],
                      )
                      nc.sync.dma_start(
                          out=wqk_sb[:, ct * 512 : (ct + 1) * 512],
                          in_=wqk_d[ct * 128 : (ct + 1) * 128, :],
                      )
                      nc.sync.dma_start(
                          out=wv_sb[:, ct * 256 : (ct + 1) * 256],
                          in_=wv_d[ct * 128 : (ct + 1) * 128, :],
                      )

                  # qkT[f] = wqk[:, f-block]^T @ xT   -> [128, S] per f-tile
                  for f in range(4):
                      qps = [
                          ps_qk.tile([128, 512], F32, tag="qkps", name=f"qps{f}_{i}")
                          for i in range(4)
                      ]
                      for ct in range(NCT):
                          for sc_ in range(4):
                              mm(
                                  qps[sc_],
                                  wqk_sb[:, ct * 512 + f * 128 : ct * 512 + (f + 1) * 128],
                                  xt_sb[:, ct * S + sc_ * 512 : ct * S + (sc_ + 1) * 512],
                                  start=(ct == 0),
                                  stop=(ct == NCT - 1),
                              )
                      for sc_ in range(4):
                          nc.vector.tensor_copy(
                              qk_sb[:, f * S + sc_ * 512 : f * S + (sc_ + 1) * 512],
                              qps[sc_],
                          )

                  # V[st] = xT[:, st-block]^T @ wv   -> [128, 256] natural layout
                  for st in range(MT):
                      vps = ps_v.tile([128, 256], F32, tag="vps")
                      for ct in range(NCT):
                          mm(
                              vps,
                              xt_sb[:, ct * S + st * 128 : ct * S + (st + 1) * 128],
                              wv_sb[:, ct * 256 : (ct + 1) * 256],
                              start=(ct == 0),
                              stop=(ct == NCT - 1),
                          )
                      nc.vector.tensor_copy(
                          vaug[:, st * 260 : (st + 1) * 260].rearrange(
                              "p (h c) -> p h c", c=65
                          )[:, :, 0:64],
                          vps.rearrange("p (h c) -> p h c", c=64),
                      )

              # ---------------- phase B/C: attention ----------------
              with (
                  tc.tile_pool(name="pt", bufs=2) as pt_pool,
                  tc.tile_pool(name="small", bufs=4) as small,
                  tc.tile_pool(name="ps_s", bufs=2, space="PSUM") as ps_s,
                  tc.tile_pool(name="ps_o", bufs=2, space="PSUM") as ps_o,
                  tc.tile_pool(name="ps_y", bufs=2, space="PSUM") as ps_y,
                  tc.tile_pool(name="yout", bufs=4) as yout,
              ):
                  groups = [(2 * g, 2) for g in range(8)]
                  for ch in range(NCH):
                    for pr in range(2):
                      qf, kf = pr, 2 + pr
                      hA, hB = 2 * pr, 2 * pr + 1
                      if True:
                          ptA = pt_pool.tile([128, MT * 512], MM_DT, tag="pt")
                          ptB = pt_pool.tile([128, MT * 512], MM_DT, tag="pt")
                          oA = ps_o.tile([128, 512], F32, tag="ops")
                          oB = ps_o.tile([128, 512], F32, tag="ops")
                          for g0, gn in groups:
                              sA = ps_s.tile([128, 1024], F32, tag="sps")
                              sB = ps_s.tile([128, 1024], F32, tag="sps")
                              for j in range(gn):
                                  m = g0 + j
                                  # two heads row-packed: A in PE rows 0-63,
                                  # B in rows 64-127 (base_partition-derived)
                                  mm(
                                      sA[:, j * 512 : (j + 1) * 512],
                                      qk_sb[0:64, kf * S + m * 128 : kf * S + (m + 1) * 128],
                                      qk_sb[0:64, qf * S + ch * 512 : qf * S + (ch + 1) * 512],
                                      start=True,
                                      stop=True,
                                  )
                                  mm(
                                      sB[:, j * 512 : (j + 1) * 512],
                                      qk_sb[64:128, kf * S + m * 128 : kf * S + (m + 1) * 128],
                                      qk_sb[64:128, qf * S + ch * 512 : qf * S + (ch + 1) * 512],
                                      start=True,
                                      stop=True,
                                  )
                              nc.scalar.activation(
                                  ptA[:, g0 * 512 : (g0 + gn) * 512],
                                  sA[:, 0 : gn * 512],
                                  EXP,
                              )
                              nc.scalar.activation(
                                  ptB[:, g0 * 512 : (g0 + gn) * 512],
                                  sB[:, 0 : gn * 512],
                                  EXP,
                              )
                              for j in range(gn):
                                  m = g0 + j
                                  mm(
                                      oA[0:65, :],
                                      vaug[:, m * 260 + 65 * hA : m * 260 + 65 * hA + 65],
                                      ptA[:, m * 512 : (m + 1) * 512],
                                      start=(m == 0),
                                      stop=(m == MT - 1),
                                  )
                                  mm(
                                      oB[0:65, :],
                                      vaug[:, m * 260 + 65 * hB : m * 260 + 65 * hB + 65],
                                      ptB[:, m * 512 : (m + 1) * 512],
                                      start=(m == 0),
                                      stop=(m == MT - 1),
                                  )
                          # normalize: row 64 of o[AB] is Z = sum_m exp(S^T)
                          for hx, ops in ((hA, oA), (hB, oB)):
                              rz = small.tile([128, 512], F32, tag="rz", name=f"rz{hx}")
                              nc.vector.reciprocal(
                                  out=rz[64:65, :], in_=ops[64:65, :]
                              )
                              bc = ps_y.tile([128, 512], F32, tag="yps", name=f"bc{hx}")
                              nc.tensor.matmul(
                                  bc[0:64, :],
                                  ones_sb[64:65, 0:64],
                                  rz[64:65, :],
                                  start=True,
                                  stop=True,
                              )
                              bcs = small.tile([128, 512], F32, tag="bcs", name=f"bcs{hx}")
                              nc.vector.tensor_copy(bcs[0:64, :], bc[0:64, :])
                              nc.vector.tensor_mul(
                                  onorm[hx][:, ch * 512 : (ch + 1) * 512],
                                  ops[0:64, :],
                                  bcs[0:64, :],
                              )

                    # ---- projection for this chunk (overlaps next chunk's exp) ----
                    for st in range(4 * ch, 4 * ch + 4):
                        for fc in range(2):
                            yps = ps_y.tile([128, 512], F32, tag="yps")
                            for h in range(HPC):
                                mm(
                                    yps,
                                    onorm[h][:, st * 128 : (st + 1) * 128],
                                    wp_sb[:, h * C + fc * 512 : h * C + (fc + 1) * 512],
                                    start=(h == 0),
                                    stop=(h == HPC - 1),
                                )
                            ysb = yout.tile([128, 512], F32, tag="ysb")
                            nc.vector.tensor_copy(ysb, yps)
                            nc.sync.dma_start(
                                out=y_d[st * 128 : (st + 1) * 128, fc * 512 : (fc + 1) * 512],
                                in_=ysb,
                            )

    nc.compile()
    return nc


def make_core_inputs(x, Wqkv, Wproj):
    """Per-core input dicts. Core c: batch c//4, heads 4*(c%4) .. 4*(c%4)+3."""
    scale = D**-0.5
    xts = [np.ascontiguousarray(x[b].T).astype(np.float32) for b in range(B)]
    in_maps = []
    for core in range(8):
        b, hg = core // 4, core % 4
        heads = [HPC * hg + i for i in range(HPC)]
        rows_q = np.concatenate([Wqkv[D * h : D * (h + 1)] for h in heads]) * scale
        rows_k = np.concatenate([Wqkv[C + D * h : C + D * (h + 1)] for h in heads])
        wqk = np.ascontiguousarray(np.concatenate([rows_q, rows_k]).T, dtype=np.float32)
        wv = np.ascontiguousarray(
            np.concatenate([Wqkv[2 * C + D * h : 2 * C + D * (h + 1)] for h in heads]).T,
            dtype=np.float32,
        )
        wp = np.ascontiguousarray(
            np.concatenate([Wproj[:, D * h : D * (h + 1)] for h in heads], axis=1).T,
            dtype=np.float32,
        )
        in_maps.append({"xt": xts[b], "wqk": wqk, "wv": wv, "wp": wp})
    return in_maps


_EXEC_CACHE = {}


def _get_executor():
    """Build + jit the 8-core SPMD executable once per process."""
    if "fn" in _EXEC_CACHE:
        return _EXEC_CACHE
    import jax
    from jax.sharding import Mesh, PartitionSpec
    from jax.experimental.shard_map import shard_map
    from concourse import bass2jax
    from concourse.bass2jax import _bass_exec_p, partition_id_tensor

    nc = build_bass()
    bass2jax.install_neuronx_cc_hook()
    pid = nc.partition_id_tensor.name if nc.partition_id_tensor else None
    in_names, out_names, out_avals = [], [], []
    for alloc in nc.m.functions[0].allocations:
        if not isinstance(alloc, mybir.MemoryLocationSet):
            continue
        name = alloc.memorylocations[0].name
        if alloc.kind == "ExternalInput":
            if name != pid:
                in_names.append(name)
        elif alloc.kind == "ExternalOutput":
            out_names.append(name)
            out_avals.append(
                jax.core.ShapedArray(
                    tuple(alloc.tensor_shape), mybir.dt.np(alloc.dtype)
                )
            )
    n_params = len(in_names)
    all_names = list(in_names) + list(out_names) + ([pid] if pid else [])

    def body(*args):
        *ins, yb = args
        operands = list(ins) + [yb]
        if pid:
            operands.append(partition_id_tensor())
        outs = _bass_exec_p.bind(
            *operands,
            out_avals=tuple(out_avals),
            in_names=tuple(all_names),
            out_names=tuple(out_names),
            lowering_input_output_aliases=(),
            sim_require_finite=True,
            sim_require_nnan=True,
            nc=nc,
        )
        return outs[0]

    mesh = Mesh(np.asarray(jax.devices()[:8]), ("core",))
    fn = jax.jit(
        shard_map(
            body,
            mesh=mesh,
            in_specs=(PartitionSpec("core"),) * (n_params + 1),
            out_specs=PartitionSpec("core"),
            check_rep=False,
        ),
        donate_argnums=(n_params,),
    )
    _EXEC_CACHE.update(fn=fn, in_names=in_names)
    return _EXEC_CACHE


def kernel(x, Wqkv, Wproj, bproj):
    x = np.asarray(x, dtype=np.float32)
    Wqkv = np.asarray(Wqkv, dtype=np.float32)
    Wproj = np.asarray(Wproj, dtype=np.float32)
    bproj = np.asarray(bproj, dtype=np.float32)

    ex = _get_executor()
    in_maps = make_core_inputs(x, Wqkv, Wproj)
    glob_ins = [
        np.concatenate([np.asarray(m[name]) for m in in_maps], axis=0)
        for name in ex["in_names"]
    ]
    y0 = np.zeros((8 * S, C), np.float32)
    out = np.asarray(ex["fn"](*glob_ins, y0))  # [8*S, C]

    y = np.zeros((B, S, C), dtype=np.float32)
    for core in range(8):
        y[core // 4] += out[core * S : (core + 1) * S, :]
    y += bproj
    return y



# revision 2
# speedup vs baseline: 1.3117x; 1.3117x over previous
"""Multi-head attention block (B=2, N=2048, C=1024, H=16, D=64) on 8 TRN2 cores.

Sharding: core c -> batch b = c // 4, head-group hg = c % 4 (4 heads per core).
All matmuls in bf16 (1 cyc/row at any moving size in the cost model).
Per core:
  qkT  = Wqk_hg @ x_b^T          (f-tiles [2 heads x 64d, 2048 tok], bf16)
  V    = x_b @ Wv_hg^T           ([tok, 4h x 64d] natural layout -> vaug [V|1])
  S^T  = kT^T q-chunks           (K=64, two heads row-packed; [128 key, 512 q])
  P^T  = exp(S^T)                (ScalarE, no max-subtraction: scores ~ N(0,1))
  O|Z  = P^T-tile^T @ [V|1]      (q-major: out [128 q, 65]; V is the moving
                                  operand -> 65 rows/pass instead of 512)
  O_n  = O * (1/Z)               (DVE reciprocal + per-partition tensor_scalar)
  O^T  = DMA-transpose(O_n)      (XBAR, SBUF->SBUF, 2-byte)
  y    = O^T^T @ Wp^T            (K=128: two heads stacked per pass)
Host sums the 4 head-group partials per batch and adds bias.

Issue order is software-pipelined: S-matmul groups for block k+1 are
interleaved with PV/normalize/proj work of block k-2 so the ScalarE exp
stream (the bottleneck engine, ~133us) never starves.
"""

import numpy as np

import concourse.bass as bass
import concourse.tile as tile
from concourse import bacc, mybir

F32 = mybir.dt.float32
BF16 = mybir.dt.bfloat16
EXP = mybir.ActivationFunctionType.Exp

B, S, C = 2, 2048, 1024
H, D = 16, 64
HPC = 4            # heads per core
NCT = C // 128     # 8 contraction tiles
MT = S // 128      # 16 key tiles
NCH = S // 512     # 4 query chunks


def build_bass(loop_n=None):
    nc = bacc.Bacc("TRN2", target_bir_lowering=False)

    xt_d = nc.dram_tensor("xt", [C, S], BF16, kind="ExternalInput")
    wqk_d = nc.dram_tensor("wqk", [C, 512], BF16, kind="ExternalInput")
    wv_d = nc.dram_tensor("wv", [C, 256], BF16, kind="ExternalInput")
    wp_d = nc.dram_tensor("wp", [256, C], BF16, kind="ExternalInput")
    y_d = nc.dram_tensor("y", [S, C], F32, kind="ExternalOutput")

    def mm(out, lhsT, rhs, start, stop):
        nc.tensor.matmul(out, lhsT, rhs, start=start, stop=stop)

    with tile.TileContext(nc) as tc:
        with (
            tc.tile_pool(name="persist", bufs=1) as persist,
            tc.tile_pool(name="pt", bufs=4) as pt_pool,
            tc.tile_pool(name="rz", bufs=4) as rz_pool,
            tc.tile_pool(name="osb", bufs=2) as osb_pool,
            tc.tile_pool(name="ysb", bufs=2) as ysb_pool,
            tc.tile_pool(name="ps_s", bufs=2, space="PSUM") as ps_s,
        ):
            # ---- persistent SBUF tiles ----
            # qkT f-tiles: 0=q_h0|q_h1, 1=q_h2|q_h3, 2=k_h0|k_h1, 3=k_h2|k_h3
            qk_sb = persist.tile([128, 4 * S], BF16, tag="qk")
            # V augmented per key tile m: [v_h0|1|v_h1|1|v_h2|1|v_h3|1]
            vaug = persist.tile([128, MT * 260], BF16, tag="vaug")
            wp_sb = persist.tile([128, 2 * C], BF16, tag="wp")
            xt_sb = persist.tile([128, NCT * S], BF16, tag="xt")
            wqk_sb = persist.tile([128, NCT * 512], BF16, tag="wqk")
            wv_sb = persist.tile([128, NCT * 256], BF16, tag="wv")
            # normalized-transposed O per (pair, ch): [128 = 2 heads x 64d, 512 q]
            onorm = [
                persist.tile([64, 0], BF16, tag="dummy")  # placeholder, replaced below
                for _ in range(0)
            ]
            onorm = {}
            for p in range(2):
                for c in range(NCH):
                    onorm[(p, c)] = persist.tile(
                        [128, 512], BF16, tag=f"onorm{p}{c}", name=f"onorm{p}{c}"
                    )

            nc.vector.memset(vaug, 1.0)
            for p in range(2):
                nc.sync.dma_start(
                    out=wp_sb[:, p * C : (p + 1) * C],
                    in_=wp_d[p * 128 : (p + 1) * 128, :],
                )
            for ct in range(NCT):
                nc.sync.dma_start(
                    out=xt_sb[:, ct * S : (ct + 1) * S],
                    in_=xt_d[ct * 128 : (ct + 1) * 128, :],
                )
                nc.sync.dma_start(
                    out=wqk_sb[:, ct * 512 : (ct + 1) * 512],
                    in_=wqk_d[ct * 128 : (ct + 1) * 128, :],
                )
                nc.sync.dma_start(
                    out=wv_sb[:, ct * 256 : (ct + 1) * 256],
                    in_=wv_d[ct * 128 : (ct + 1) * 128, :],
                )

            # ---------------- step generators ----------------

            def qk_f_steps(f, ps_a):
                """Compute qkT f-tile f: out qk_sb[:, f*S:(f+1)*S]. Two steps
                (sc pairs), ct-outer accumulation inside each."""
                for scp in range(2):
                    def step(f=f, scp=scp):
                        qps = [
                            ps_a.tile([128, 512], F32, tag="psa",
                                      name=f"qps{f}_{scp}_{i}")
                            for i in range(2)
                        ]
                        for ct in range(NCT):
                            for j in range(2):
                                sc = 2 * scp + j
                                mm(
                                    qps[j],
                                    wqk_sb[:, ct * 512 + f * 128 : ct * 512 + (f + 1) * 128],
                                    xt_sb[:, ct * S + sc * 512 : ct * S + (sc + 1) * 512],
                                    start=(ct == 0),
                                    stop=(ct == NCT - 1),
                                )
                        for j in range(2):
                            sc = 2 * scp + j
                            nc.vector.tensor_copy(
                                qk_sb[:, f * S + sc * 512 : f * S + (sc + 1) * 512],
                                qps[j],
                            )
                    yield step

            def v_steps(m0, m1, ps_a):
                """V key-tiles m0..m1-1 -> vaug (strided, ones preserved)."""
                for m in range(m0, m1):
                    def step(m=m):
                        vps = ps_a.tile([128, 256], F32, tag="psa", name=f"vps{m}")
                        for ct in range(NCT):
                            mm(
                                vps,
                                xt_sb[:, ct * S + m * 128 : ct * S + (m + 1) * 128],
                                wv_sb[:, ct * 256 : (ct + 1) * 256],
                                start=(ct == 0),
                                stop=(ct == NCT - 1),
                            )
                        nc.vector.tensor_copy(
                            vaug[:, m * 260 : (m + 1) * 260].rearrange(
                                "p (h c) -> p h c", c=65
                            )[:, :, 0:64],
                            vps.rearrange("p (h c) -> p h c", c=64),
                        )
                    yield step

            def s_steps(pair, ch, pts):
                """Score+exp stream for block (pair, ch): 8 mg steps, each
                doing head A and head B over key tiles 2mg, 2mg+1."""
                qf, kf = pair, 2 + pair
                ptA = pt_pool.tile([128, MT * 512], BF16, tag="pt",
                                   name=f"ptA_{pair}_{ch}")
                ptB = pt_pool.tile([128, MT * 512], BF16, tag="pt",
                                   name=f"ptB_{pair}_{ch}")
                pts[(pair, ch)] = (ptA, ptB)
                for mg in range(MT // 2):
                    def step(pair=pair, ch=ch, mg=mg, ptA=ptA, ptB=ptB):
                        qf, kf = pair, 2 + pair
                        for hh, pt in ((0, ptA), (1, ptB)):
                            lo, hi = 64 * hh, 64 * hh + 64
                            sps = ps_s.tile([128, 1024], F32, tag="sps",
                                            name=f"s{pair}{ch}{mg}{hh}")
                            for j in range(2):
                                m = 2 * mg + j
                                mm(
                                    sps[:, j * 512 : (j + 1) * 512],
                                    qk_sb[lo:hi, kf * S + m * 128 : kf * S + (m + 1) * 128],
                                    qk_sb[lo:hi, qf * S + ch * 512 : qf * S + (ch + 1) * 512],
                                    start=True,
                                    stop=True,
                                )
                            nc.scalar.activation(
                                pt[:, 2 * mg * 512 : (2 * mg + 2) * 512],
                                sps,
                                EXP,
                            )
                    yield step

            def work_steps(pair, ch, pts, ps_o, ps_y):
                """PV + normalize + transpose for block (pair, ch); proj for
                ch when pair==1. 4 qt steps + (4 proj steps if pair 1)."""
                ptA, ptB = pts[(pair, ch)]
                for qt in range(4):
                    def step(pair=pair, ch=ch, qt=qt, ptA=ptA, ptB=ptB):
                        osb = osb_pool.tile([128, 128], BF16, tag="osb",
                                            name=f"osb{pair}{ch}{qt}")
                        for hh, pt in ((0, ptA), (1, ptB)):
                            h = 2 * pair + hh
                            ops = ps_o.tile([128, 65], F32, tag="ops",
                                            name=f"o{pair}{ch}{qt}{hh}")
                            for m in range(MT):
                                mm(
                                    ops,
                                    pt[:, m * 512 + qt * 128 : m * 512 + qt * 128 + 128],
                                    vaug[:, m * 260 + h * 65 : m * 260 + h * 65 + 65],
                                    start=(m == 0),
                                    stop=(m == MT - 1),
                                )
                            rz = rz_pool.tile([128, 1], F32, tag="rz",
                                              name=f"rz{pair}{ch}{qt}{hh}")
                            nc.vector.reciprocal(out=rz, in_=ops[:, 64:65])
                            nc.vector.tensor_scalar_mul(
                                osb[:, hh * 64 : hh * 64 + 64],
                                ops[:, 0:64],
                                rz,
                            )
                        # full [128 q, 128 d-pair] -> [128 d-pair, 128 q]
                        nc.sync.dma_start(
                            out=onorm[(pair, ch)][:, qt * 128 : (qt + 1) * 128],
                            in_=osb,
                            transpose=True,
                        )
                    yield step
                if pair == 1:
                    for qt in range(4):
                        def step(ch=ch, qt=qt):
                            st = ch * 4 + qt
                            ysb = ysb_pool.tile([128, C], F32, tag="ysb",
                                                name=f"ysb{ch}{qt}")
                            for fc in range(2):
                                yps = ps_y.tile([128, 512], F32, tag="yps",
                                                name=f"y{ch}{qt}{fc}")
                                for p in range(2):
                                    mm(
                                        yps,
                                        onorm[(p, ch)][:, qt * 128 : (qt + 1) * 128],
                                        wp_sb[:, p * C + fc * 512 : p * C + (fc + 1) * 512],
                                        start=(p == 0),
                                        stop=(p == 1),
                                    )
                                nc.gpsimd.tensor_copy(
                                    ysb[:, fc * 512 : (fc + 1) * 512], yps
                                )
                            nc.sync.dma_start(
                                out=y_d[st * 128 : (st + 1) * 128, :], in_=ysb
                            )
                        yield step

            def interleave(*gens):
                gens = [g for g in gens if g is not None]
                done = False
                while not done:
                    done = True
                    for g in gens:
                        step = next(g, None)
                        if step is not None:
                            step()
                            done = False

            def drain(g):
                if g is None:
                    return
                for step in g:
                    step()

            # ---------------- pipelined schedule ----------------
            blocks = [(p, c) for p in range(2) for c in range(NCH)]
            pts = {}

            with tc.tile_pool(name="ps_a", bufs=2, space="PSUM") as ps_a:
                # pre-windows: k-heads pair0 then q pair0 (gets exp started)
                drain(qk_f_steps(2, ps_a))
                drain(qk_f_steps(0, ps_a))
                # window 0: S(p0,c0) || f3, f1
                interleave(
                    s_steps(0, 0, pts),
                    qk_f_steps(3, ps_a),
                    qk_f_steps(1, ps_a),
                )
                # window 1: S(p0,c1) || V(all 16 key tiles)
                interleave(
                    s_steps(0, 1, pts),
                    v_steps(0, MT, ps_a),
                )

            with (
                tc.tile_pool(name="ps_o", bufs=2, space="PSUM") as ps_o,
                tc.tile_pool(name="ps_y", bufs=2, space="PSUM") as ps_y,
            ):
                # windows 2..7: S(block k) || work(block k-2)
                for k in range(2, len(blocks)):
                    interleave(
                        s_steps(*blocks[k], pts),
                        work_steps(*blocks[k - 2], pts, ps_o, ps_y),
                    )
                # tail: work for the last two blocks
                drain(work_steps(*blocks[-2], pts, ps_o, ps_y))
                drain(work_steps(*blocks[-1], pts, ps_o, ps_y))

    nc.compile()
    return nc


def make_core_inputs(x, Wqkv, Wproj):
    """Per-core input dicts. Core c: batch c//4, heads 4*(c%4) .. 4*(c%4)+3."""
    bf16 = mybir.dt.np(BF16)
    scale = D**-0.5  # 1/8, exact in bf16
    xts = [np.ascontiguousarray(x[b].T).astype(bf16) for b in range(B)]
    in_maps = []
    for core in range(8):
        b, hg = core // 4, core % 4
        heads = [HPC * hg + i for i in range(HPC)]
        rows_q = np.concatenate([Wqkv[D * h : D * (h + 1)] for h in heads]) * scale
        rows_k = np.concatenate([Wqkv[C + D * h : C + D * (h + 1)] for h in heads])
        wqk = np.ascontiguousarray(np.concatenate([rows_q, rows_k]).T).astype(bf16)
        wv = np.ascontiguousarray(
            np.concatenate([Wqkv[2 * C + D * h : 2 * C + D * (h + 1)] for h in heads]).T
        ).astype(bf16)
        wp = np.ascontiguousarray(
            np.concatenate([Wproj[:, D * h : D * (h + 1)] for h in heads], axis=1).T
        ).astype(bf16)
        in_maps.append({"xt": xts[b], "wqk": wqk, "wv": wv, "wp": wp})
    return in_maps


_EXEC_CACHE = {}


def _get_executor():
    """Build + jit the 8-core SPMD executable once per process."""
    if "fn" in _EXEC_CACHE:
        return _EXEC_CACHE
    import jax
    from jax.sharding import Mesh, PartitionSpec
    from jax.experimental.shard_map import shard_map
    from concourse import bass2jax
    from concourse.bass2jax import _bass_exec_p, partition_id_tensor

    nc = build_bass()
    bass2jax.install_neuronx_cc_hook()
    pid = nc.partition_id_tensor.name if nc.partition_id_tensor else None
    in_names, out_names, out_avals = [], [], []
    for alloc in nc.m.functions[0].allocations:
        if not isinstance(alloc, mybir.MemoryLocationSet):
            continue
        name = alloc.memorylocations[0].name
        if alloc.kind == "ExternalInput":
            if name != pid:
                in_names.append(name)
        elif alloc.kind == "ExternalOutput":
            out_names.append(name)
            out_avals.append(
                jax.core.ShapedArray(
                    tuple(alloc.tensor_shape), mybir.dt.np(alloc.dtype)
                )
            )
    n_params = len(in_names)
    all_names = list(in_names) + list(out_names) + ([pid] if pid else [])

    def body(*args):
        *ins, yb = args
        operands = list(ins) + [yb]
        if pid:
            operands.append(partition_id_tensor())
        outs = _bass_exec_p.bind(
            *operands,
            out_avals=tuple(out_avals),
            in_names=tuple(all_names),
            out_names=tuple(out_names),
            lowering_input_output_aliases=(),
            sim_require_finite=True,
            sim_require_nnan=True,
            nc=nc,
        )
        return outs[0]

    mesh = Mesh(np.asarray(jax.devices()[:8]), ("core",))
    fn = jax.jit(
        shard_map(
            body,
            mesh=mesh,
            in_specs=(PartitionSpec("core"),) * (n_params + 1),
            out_specs=PartitionSpec("core"),
            check_rep=False,
        ),
        donate_argnums=(n_params,),
    )
    _EXEC_CACHE.update(fn=fn, in_names=in_names)
    return _EXEC_CACHE


def kernel(x, Wqkv, Wproj, bproj):
    x = np.asarray(x, dtype=np.float32)
    Wqkv = np.asarray(Wqkv, dtype=np.float32)
    Wproj = np.asarray(Wproj, dtype=np.float32)
    bproj = np.asarray(bproj, dtype=np.float32)

    ex = _get_executor()
    in_maps = make_core_inputs(x, Wqkv, Wproj)
    glob_ins = [
        np.concatenate([np.asarray(m[name]) for m in in_maps], axis=0)
        for name in ex["in_names"]
    ]
    y0 = np.zeros((8 * S, C), np.float32)
    out = np.asarray(ex["fn"](*glob_ins, y0))  # [8*S, C]

    y = np.zeros((B, S, C), dtype=np.float32)
    for core in range(8):
        y[core // 4] += out[core * S : (core + 1) * S, :]
    y += bproj
    return y


# revision 11
# speedup vs baseline: 1.3962x; 1.0644x over previous
"""Multi-head attention block (B=2, N=2048, C=1024, H=16, D=64) on 8 TRN2 cores.

Sharding: core c -> batch b = c // 4, head-group hg = c % 4 (4 heads per core).
All matmuls in bf16 (1 cyc/row at any moving size in the cost model).
Per core:
  qkT  = Wqk_hg @ x_b^T          (f-tiles [2 heads x 64d, 2048 tok], bf16)
  V    = x_b @ Wv_hg^T           ([tok, 4h x 64d] natural layout -> vaug [V|1])
  S^T  = kT^T q-chunks           (K=64, two heads row-packed; [128 key, 512 q])
  P^T  = exp(S^T)                (ScalarE, no max-subtraction: scores ~ N(0,1))
  O|Z  = P^T-tile^T @ [V|1]      (q-major: out [128 q, 65]; V is the moving
                                  operand -> 65 rows/pass instead of 512)
  O_n  = O * (1/Z)               (DVE reciprocal + per-partition tensor_scalar)
  O^T  = DMA-transpose(O_n)      (XBAR, SBUF->SBUF, 2-byte)
  y    = O^T^T @ Wp^T            (K=128: two heads stacked per pass)
Host sums the 4 head-group partials per batch and adds bias.

Issue order is software-pipelined: S-matmul groups for block k+1 are
interleaved with PV/normalize/proj work of block k-2 so the ScalarE exp
stream (the bottleneck engine, ~133us) never starves.
"""

import numpy as np

import concourse.bass as bass
import concourse.tile as tile
from concourse import bacc, mybir

F32 = mybir.dt.float32
BF16 = mybir.dt.bfloat16
EXP = mybir.ActivationFunctionType.Exp

B, S, C = 2, 2048, 1024
H, D = 16, 64
HPC = 4            # heads per core
NCT = C // 128     # 8 contraction tiles
MT = S // 128      # 16 key tiles
NCH = S // 512     # 4 query chunks


def build_bass(loop_n=None):
    nc = bacc.Bacc("TRN2", target_bir_lowering=False)

    xt_d = nc.dram_tensor("xt", [C, S], BF16, kind="ExternalInput")
    wqk_d = nc.dram_tensor("wqk", [C, 512], BF16, kind="ExternalInput")
    wv_d = nc.dram_tensor("wv", [C, 256], BF16, kind="ExternalInput")
    wp_d = nc.dram_tensor("wp", [256, C], BF16, kind="ExternalInput")
    y_d = nc.dram_tensor("y", [S, C], F32, kind="ExternalOutput")

    def mm(out, lhsT, rhs, start, stop):
        nc.tensor.matmul(out, lhsT, rhs, start=start, stop=stop)

    with tile.TileContext(nc) as tc:
        with (
            tc.tile_pool(name="persist", bufs=1) as persist,
            tc.tile_pool(name="pt", bufs=4) as pt_pool,
            tc.tile_pool(name="rz", bufs=4) as rz_pool,
            tc.tile_pool(name="osb", bufs=2) as osb_pool,
            tc.tile_pool(name="ysb", bufs=2) as ysb_pool,
            tc.tile_pool(name="ps_s", bufs=2, space="PSUM") as ps_s,
        ):
            # ---- persistent SBUF tiles ----
            # qkT f-tiles: 0=q_h0|q_h1, 1=q_h2|q_h3, 2=k_h0|k_h1, 3=k_h2|k_h3
            qk_sb = persist.tile([128, 4 * S], BF16, tag="qk")
            # V augmented per key tile m: [v_h0|1|v_h1|1|v_h2|1|v_h3|1]
            vaug = persist.tile([128, MT * 260], BF16, tag="vaug")
            wp_sb = persist.tile([128, 2 * C], BF16, tag="wp")
            xt_sb = persist.tile([128, NCT * S], BF16, tag="xt")
            wqk_sb = persist.tile([128, NCT * 512], BF16, tag="wqk")
            wv_sb = persist.tile([128, NCT * 256], BF16, tag="wv")
            # normalized-transposed O per (pair, ch): [128 = 2 heads x 64d, 512 q]
            onorm = [
                persist.tile([64, 0], BF16, tag="dummy")  # placeholder, replaced below
                for _ in range(0)
            ]
            onorm = {}
            for p in range(2):
                for c in range(NCH):
                    onorm[(p, c)] = persist.tile(
                        [128, 512], BF16, tag=f"onorm{p}{c}", name=f"onorm{p}{c}"
                    )

            nc.vector.memset(vaug, 1.0)

            def dma_wqk_f(f):
                # wqk_sb layout: [128, f*1024 + ct*128 + j]
                nc.sync.dma_start(
                    out=wqk_sb[:, f * 1024 : (f + 1) * 1024].rearrange(
                        "p (c j) -> p c j", c=NCT
                    ),
                    in_=wqk_d[:, f * 128 : (f + 1) * 128].rearrange(
                        "(c p) j -> p c j", p=128
                    ),
                )

            # DMA priority order: k-pair0 weights, then x, then the rest.
            dma_wqk_f(2)
            for ct in range(NCT):
                nc.sync.dma_start(
                    out=xt_sb[:, ct * S : (ct + 1) * S],
                    in_=xt_d[ct * 128 : (ct + 1) * 128, :],
                )
            for f in (0, 3, 1):
                dma_wqk_f(f)
            for ct in range(NCT):
                nc.sync.dma_start(
                    out=wv_sb[:, ct * 256 : (ct + 1) * 256],
                    in_=wv_d[ct * 128 : (ct + 1) * 128, :],
                )
            for p in range(2):
                nc.sync.dma_start(
                    out=wp_sb[:, p * C : (p + 1) * C],
                    in_=wp_d[p * 128 : (p + 1) * 128, :],
                )

            # ---------------- step generators ----------------

            def qk_f_block(f, ps_a, scs=range(4)):
                """qkT f-tile f, all scs ct-outer concurrently (needs
                len(scs) free ps_a bufs). Used during the input DMA stream."""
                qps = {
                    sc: ps_a.tile([128, 512], F32, tag="psa", name=f"qps{f}_{sc}")
                    for sc in scs
                }
                for ct in range(NCT):
                    for sc in scs:
                        mm(
                            qps[sc],
                            wqk_sb[:, f * 1024 + ct * 128 : f * 1024 + (ct + 1) * 128],
                            xt_sb[:, ct * S + sc * 512 : ct * S + (sc + 1) * 512],
                            start=(ct == 0),
                            stop=(ct == NCT - 1),
                        )
                for sc in scs:
                    nc.vector.tensor_copy(
                        qk_sb[:, f * S + sc * 512 : f * S + (sc + 1) * 512],
                        qps[sc],
                    )

            def qk_f_steps(f, ps_a, scs=range(4)):
                """Single-sc ct-inner steps (~1.7us each) for filler use once
                the input DMAs have landed."""
                for sc in scs:
                    def step(f=f, sc=sc):
                        qps = ps_a.tile([128, 512], F32, tag="psa",
                                        name=f"qps{f}_{sc}")
                        for ct in range(NCT):
                            mm(
                                qps,
                                wqk_sb[:, f * 1024 + ct * 128 : f * 1024 + (ct + 1) * 128],
                                xt_sb[:, ct * S + sc * 512 : ct * S + (sc + 1) * 512],
                                start=(ct == 0),
                                stop=(ct == NCT - 1),
                            )
                        nc.vector.tensor_copy(
                            qk_sb[:, f * S + sc * 512 : f * S + (sc + 1) * 512],
                            qps,
                        )
                    yield step

            def v_steps(m0, m1, pool, tag):
                """V key-tiles m0..m1-1 -> vaug (strided, ones preserved)."""
                for m in range(m0, m1):
                    def step(m=m, pool=pool, tag=tag):
                        vps_full = pool.tile([128, 512], F32, tag=tag,
                                             name=f"vps{m}")
                        vps = vps_full[:, 0:256]
                        for ct in range(NCT):
                            mm(
                                vps,
                                xt_sb[:, ct * S + m * 128 : ct * S + (m + 1) * 128],
                                wv_sb[:, ct * 256 : (ct + 1) * 256],
                                start=(ct == 0),
                                stop=(ct == NCT - 1),
                            )
                        nc.vector.tensor_copy(
                            vaug[:, m * 260 : (m + 1) * 260].rearrange(
                                "p (h c) -> p h c", c=65
                            )[:, :, 0:64],
                            vps.rearrange("p (h c) -> p h c", c=64),
                        )
                    yield step

            def s_steps(pair, ch, pts):
                """Score+exp stream for block (pair, ch): 8 mg steps, each
                doing head A and head B over key tiles 2mg, 2mg+1."""
                qf, kf = pair, 2 + pair
                ptA = pt_pool.tile([128, MT * 512], BF16, tag="pt",
                                   name=f"ptA_{pair}_{ch}")
                ptB = pt_pool.tile([128, MT * 512], BF16, tag="pt",
                                   name=f"ptB_{pair}_{ch}")
                pts[(pair, ch)] = (ptA, ptB)
                for mg in range(MT // 2):
                    def step(pair=pair, ch=ch, mg=mg, ptA=ptA, ptB=ptB):
                        qf, kf = pair, 2 + pair
                        for hh, pt in ((0, ptA), (1, ptB)):
                            lo, hi = 64 * hh, 64 * hh + 64
                            sps = ps_s.tile([128, 1024], F32, tag="sps",
                                            name=f"s{pair}{ch}{mg}{hh}")
                            for j in range(2):
                                m = 2 * mg + j
                                mm(
                                    sps[:, j * 512 : (j + 1) * 512],
                                    qk_sb[lo:hi, kf * S + m * 128 : kf * S + (m + 1) * 128],
                                    qk_sb[lo:hi, qf * S + ch * 512 : qf * S + (ch + 1) * 512],
                                    start=True,
                                    stop=True,
                                )
                            nc.scalar.activation(
                                pt[:, 2 * mg * 512 : (2 * mg + 2) * 512],
                                sps,
                                EXP,
                            )
                    yield step

            def work_steps(pair, ch, pts, ps_oy):
                """PV + normalize + transpose for block (pair, ch); when
                pair==1 also project+store token block qt of chunk ch."""
                ptA, ptB = pts[(pair, ch)]
                for qt in range(4):
                    def step(pair=pair, ch=ch, qt=qt, ptA=ptA, ptB=ptB):
                        osb = osb_pool.tile([128, 128], BF16, tag="osb",
                                            name=f"osb{pair}{ch}{qt}")
                        for hh, pt in ((0, ptA), (1, ptB)):
                            h = 2 * pair + hh
                            ops = ps_oy.tile([128, 512], F32, tag="psoy",
                                             name=f"o{pair}{ch}{qt}{hh}")
                            for m in range(MT):
                                mm(
                                    ops[:, 0:65],
                                    pt[:, m * 512 + qt * 128 : m * 512 + qt * 128 + 128],
                                    vaug[:, m * 260 + h * 65 : m * 260 + h * 65 + 65],
                                    start=(m == 0),
                                    stop=(m == MT - 1),
                                )
                            rz = rz_pool.tile([128, 1], F32, tag="rz",
                                              name=f"rz{pair}{ch}{qt}{hh}")
                            nc.vector.reciprocal(out=rz, in_=ops[:, 64:65])
                            nc.vector.tensor_scalar_mul(
                                osb[:, hh * 64 : hh * 64 + 64],
                                ops[:, 0:64],
                                rz,
                            )
                        # full [128 q, 128 d-pair] -> [128 d-pair, 128 q]
                        nc.sync.dma_start(
                            out=onorm[(pair, ch)][:, qt * 128 : (qt + 1) * 128],
                            in_=osb,
                            transpose=True,
                        )
                    yield step
                    if pair == 1:
                        def pstep(ch=ch, qt=qt):
                            st = ch * 4 + qt
                            ysb = ysb_pool.tile([128, C], F32, tag="ysb",
                                                name=f"ysb{ch}{qt}")
                            for fc in range(2):
                                yps = ps_oy.tile([128, 512], F32, tag="psoy",
                                                 name=f"y{ch}{qt}{fc}")
                                for p in range(2):
                                    mm(
                                        yps,
                                        onorm[(p, ch)][:, qt * 128 : (qt + 1) * 128],
                                        wp_sb[:, p * C + fc * 512 : p * C + (fc + 1) * 512],
                                        start=(p == 0),
                                        stop=(p == 1),
                                    )
                                nc.gpsimd.tensor_copy(
                                    ysb[:, fc * 512 : (fc + 1) * 512], yps
                                )
                            nc.sync.dma_start(
                                out=y_d[st * 128 : (st + 1) * 128, :], in_=ysb
                            )
                        yield pstep

            def chain(*gens):
                for g in gens:
                    yield from g

            def interleave(sgen, fgen):
                """Alternate one S step with one filler step; drain leftovers."""
                while True:
                    s = next(sgen, None)
                    if s is not None:
                        s()
                    f = next(fgen, None)
                    if f is not None:
                        f()
                    if s is None and f is None:
                        return

            def drain(g):
                for step in g:
                    step()

            # ---------------- pipelined schedule ----------------
            blocks = [(p, c) for p in range(2) for c in range(NCH)]
            pts = {}

            with tc.tile_pool(name="ps_a", bufs=4, space="PSUM") as ps_a:
                # during the input DMA stream: k-pair0 (4 sc ct-outer), then
                # the q columns S(p0,c0) needs
                qk_f_block(2, ps_a)
                drain(qk_f_steps(0, ps_a, scs=(0,)))
                # window 0: S(p0,c0) || remaining q0 + k1 columns
                interleave(
                    s_steps(0, 0, pts),
                    chain(
                        qk_f_steps(0, ps_a, scs=(1, 2, 3)),
                        qk_f_steps(3, ps_a),
                        qk_f_steps(1, ps_a, scs=(0,)),
                    ),
                )
                # window 1: S(p0,c1) || q1 rest + V m0..4
                interleave(
                    s_steps(0, 1, pts),
                    chain(
                        qk_f_steps(1, ps_a, scs=(1, 2, 3)),
                        v_steps(0, 5, ps_a, 'psa'),
                    ),
                )
            with tc.tile_pool(name="ps_oy", bufs=4, space="PSUM") as ps_oy:
                # window 2: S(p0,c2) || V m5..15 then work(B0)
                interleave(
                    s_steps(0, 2, pts),
                    chain(v_steps(5, MT, ps_oy, 'psoy'),
                          work_steps(*blocks[0], pts, ps_oy)),
                )
                # windows 3..7: S(block k) || work(block k-2)
                for k in range(3, len(blocks)):
                    interleave(
                        s_steps(*blocks[k], pts),
                        work_steps(*blocks[k - 2], pts, ps_oy),
                    )
                # tail
                for k in (len(blocks) - 2, len(blocks) - 1):
                    drain(work_steps(*blocks[k], pts, ps_oy))

    nc.compile()
    return nc


def make_core_inputs(x, Wqkv, Wproj):
    """Per-core input dicts. Core c: batch c//4, heads 4*(c%4) .. 4*(c%4)+3."""
    bf16 = mybir.dt.np(BF16)
    scale = D**-0.5  # 1/8, exact in bf16
    xts = [np.ascontiguousarray(x[b].T).astype(bf16) for b in range(B)]
    in_maps = []
    for core in range(8):
        b, hg = core // 4, core % 4
        heads = [HPC * hg + i for i in range(HPC)]
        rows_q = np.concatenate([Wqkv[D * h : D * (h + 1)] for h in heads]) * scale
        rows_k = np.concatenate([Wqkv[C + D * h : C + D * (h + 1)] for h in heads])
        wqk = np.ascontiguousarray(np.concatenate([rows_q, rows_k]).T).astype(bf16)
        wv = np.ascontiguousarray(
            np.concatenate([Wqkv[2 * C + D * h : 2 * C + D * (h + 1)] for h in heads]).T
        ).astype(bf16)
        wp = np.ascontiguousarray(
            np.concatenate([Wproj[:, D * h : D * (h + 1)] for h in heads], axis=1).T
        ).astype(bf16)
        in_maps.append({"xt": xts[b], "wqk": wqk, "wv": wv, "wp": wp})
    return in_maps


_EXEC_CACHE = {}


def _get_executor():
    """Build + jit the 8-core SPMD executable once per process."""
    if "fn" in _EXEC_CACHE:
        return _EXEC_CACHE
    import jax
    from jax.sharding import Mesh, PartitionSpec
    from jax.experimental.shard_map import shard_map
    from concourse import bass2jax
    from concourse.bass2jax import _bass_exec_p, partition_id_tensor

    nc = build_bass()
    bass2jax.install_neuronx_cc_hook()
    pid = nc.partition_id_tensor.name if nc.partition_id_tensor else None
    in_names, out_names, out_avals = [], [], []
    for alloc in nc.m.functions[0].allocations:
        if not isinstance(alloc, mybir.MemoryLocationSet):
            continue
        name = alloc.memorylocations[0].name
        if alloc.kind == "ExternalInput":
            if name != pid:
                in_names.append(name)
        elif alloc.kind == "ExternalOutput":
            out_names.append(name)
            out_avals.append(
                jax.core.ShapedArray(
                    tuple(alloc.tensor_shape), mybir.dt.np(alloc.dtype)
                )
            )
    n_params = len(in_names)
    all_names = list(in_names) + list(out_names) + ([pid] if pid else [])

    def body(*args):
        *ins, yb = args
        operands = list(ins) + [yb]
        if pid:
            operands.append(partition_id_tensor())
        outs = _bass_exec_p.bind(
            *operands,
            out_avals=tuple(out_avals),
            in_names=tuple(all_names),
            out_names=tuple(out_names),
            lowering_input_output_aliases=(),
            sim_require_finite=True,
            sim_require_nnan=True,
            nc=nc,
        )
        return outs[0]

    mesh = Mesh(np.asarray(jax.devices()[:8]), ("core",))
    fn = jax.jit(
        shard_map(
            body,
            mesh=mesh,
            in_specs=(PartitionSpec("core"),) * (n_params + 1),
            out_specs=PartitionSpec("core"),
            check_rep=False,
        ),
        donate_argnums=(n_params,),
    )
    _EXEC_CACHE.update(fn=fn, in_names=in_names)
    return _EXEC_CACHE


def kernel(x, Wqkv, Wproj, bproj):
    x = np.asarray(x, dtype=np.float32)
    Wqkv = np.asarray(Wqkv, dtype=np.float32)
    Wproj = np.asarray(Wproj, dtype=np.float32)
    bproj = np.asarray(bproj, dtype=np.float32)

    ex = _get_executor()
    in_maps = make_core_inputs(x, Wqkv, Wproj)
    glob_ins = [
        np.concatenate([np.asarray(m[name]) for m in in_maps], axis=0)
        for name in ex["in_names"]
    ]
    y0 = np.zeros((8 * S, C), np.float32)
    out = np.asarray(ex["fn"](*glob_ins, y0))  # [8*S, C]

    y = np.zeros((B, S, C), dtype=np.float32)
    for core in range(8):
        y[core // 4] += out[core * S : (core + 1) * S, :]
    y += bproj
    return y


# revision 12
# speedup vs baseline: 1.3977x; 1.0011x over previous
"""Multi-head attention block (B=2, N=2048, C=1024, H=16, D=64) on 8 TRN2 cores.

Sharding: core c -> batch b = c // 4, head-group hg = c % 4 (4 heads per core).
All matmuls in bf16 (1 cyc/row at any moving size in the cost model).
Per core:
  qkT  = Wqk_hg @ x_b^T          (f-tiles [2 heads x 64d, 2048 tok], bf16)
  V    = x_b @ Wv_hg^T           ([tok, 4h x 64d] natural layout -> vaug [V|1])
  S^T  = kT^T q-chunks           (K=64, two heads row-packed; [128 key, 512 q])
  P^T  = exp(S^T)                (ScalarE, no max-subtraction: scores ~ N(0,1))
  O|Z  = P^T-tile^T @ [V|1]      (q-major: out [128 q, 65]; V is the moving
                                  operand -> 65 rows/pass instead of 512)
  O_n  = O * (1/Z)               (DVE reciprocal + per-partition tensor_scalar)
  O^T  = DMA-transpose(O_n)      (XBAR, SBUF->SBUF, 2-byte)
  y    = O^T^T @ Wp^T            (K=128: two heads stacked per pass)
Host sums the 4 head-group partials per batch and adds bias.

Issue order is software-pipelined: S-matmul groups for block k+1 are
interleaved with PV/normalize/proj work of block k-2 so the ScalarE exp
stream (the bottleneck engine, ~133us) never starves.
"""

import numpy as np

import concourse.bass as bass
import concourse.tile as tile
from concourse import bacc, mybir

F32 = mybir.dt.float32
BF16 = mybir.dt.bfloat16
EXP = mybir.ActivationFunctionType.Exp

B, S, C = 2, 2048, 1024
H, D = 16, 64
HPC = 4            # heads per core
NCT = C // 128     # 8 contraction tiles
MT = S // 128      # 16 key tiles
NCH = S // 512     # 4 query chunks


def build_bass(loop_n=None):
    nc = bacc.Bacc("TRN2", target_bir_lowering=False)

    xt_d = nc.dram_tensor("xt", [C, S], BF16, kind="ExternalInput")
    wqk_d = nc.dram_tensor("wqk", [C, 512], BF16, kind="ExternalInput")
    wv_d = nc.dram_tensor("wv", [C, 256], BF16, kind="ExternalInput")
    wp_d = nc.dram_tensor("wp", [256, C], BF16, kind="ExternalInput")
    y_d = nc.dram_tensor("y", [S, C], F32, kind="ExternalOutput")

    def mm(out, lhsT, rhs, start, stop):
        nc.tensor.matmul(out, lhsT, rhs, start=start, stop=stop)

    with tile.TileContext(nc) as tc:
        with (
            tc.tile_pool(name="persist", bufs=1) as persist,
            tc.tile_pool(name="pt", bufs=4) as pt_pool,
            tc.tile_pool(name="rz", bufs=4) as rz_pool,
            tc.tile_pool(name="osb", bufs=2) as osb_pool,
            tc.tile_pool(name="ysb", bufs=2) as ysb_pool,
            tc.tile_pool(name="ps_s", bufs=2, space="PSUM") as ps_s,
            tc.tile_pool(name="ps_w", bufs=4, space="PSUM") as ps_w,
        ):
            # ---- persistent SBUF tiles ----
            # qkT f-tiles: 0=q_h0|q_h1, 1=q_h2|q_h3, 2=k_h0|k_h1, 3=k_h2|k_h3
            qk_sb = persist.tile([128, 4 * S], BF16, tag="qk")
            # V augmented per key tile m: [v_h0|1|v_h1|1|v_h2|1|v_h3|1]
            vaug = persist.tile([128, MT * 260], BF16, tag="vaug")
            wp_sb = persist.tile([128, 2 * C], BF16, tag="wp")
            xt_sb = persist.tile([128, NCT * S], BF16, tag="xt")
            wqk_sb = persist.tile([128, NCT * 512], BF16, tag="wqk")
            wv_sb = persist.tile([128, NCT * 256], BF16, tag="wv")
            # normalized-transposed O per (pair, ch): [128 = 2 heads x 64d, 512 q]
            onorm = [
                persist.tile([64, 0], BF16, tag="dummy")  # placeholder, replaced below
                for _ in range(0)
            ]
            onorm = {}
            for p in range(2):
                for c in range(NCH):
                    onorm[(p, c)] = persist.tile(
                        [128, 512], BF16, tag=f"onorm{p}{c}", name=f"onorm{p}{c}"
                    )

            nc.vector.memset(vaug, 1.0)

            def dma_wqk_f(f):
                # wqk_sb layout: [128, f*1024 + ct*128 + j]
                nc.sync.dma_start(
                    out=wqk_sb[:, f * 1024 : (f + 1) * 1024].rearrange(
                        "p (c j) -> p c j", c=NCT
                    ),
                    in_=wqk_d[:, f * 128 : (f + 1) * 128].rearrange(
                        "(c p) j -> p c j", p=128
                    ),
                )

            # DMA priority order: k-pair0 weights, then x, then the rest.
            dma_wqk_f(2)
            for ct in range(NCT):
                nc.sync.dma_start(
                    out=xt_sb[:, ct * S : (ct + 1) * S],
                    in_=xt_d[ct * 128 : (ct + 1) * 128, :],
                )
            for f in (0, 3, 1):
                dma_wqk_f(f)
            for ct in range(NCT):
                nc.sync.dma_start(
                    out=wv_sb[:, ct * 256 : (ct + 1) * 256],
                    in_=wv_d[ct * 128 : (ct + 1) * 128, :],
                )
            for p in range(2):
                nc.sync.dma_start(
                    out=wp_sb[:, p * C : (p + 1) * C],
                    in_=wp_d[p * 128 : (p + 1) * 128, :],
                )

            # ---------------- step generators ----------------

            def qk_f_block(f, ps_a, scs=range(4)):
                """qkT f-tile f, all scs ct-outer concurrently (needs
                len(scs) free ps_a bufs). Used during the input DMA stream."""
                qps = {
                    sc: ps_a.tile([128, 512], F32, tag="psw", name=f"qps{f}_{sc}")
                    for sc in scs
                }
                for ct in range(NCT):
                    for sc in scs:
                        mm(
                            qps[sc],
                            wqk_sb[:, f * 1024 + ct * 128 : f * 1024 + (ct + 1) * 128],
                            xt_sb[:, ct * S + sc * 512 : ct * S + (sc + 1) * 512],
                            start=(ct == 0),
                            stop=(ct == NCT - 1),
                        )
                for sc in scs:
                    nc.vector.tensor_copy(
                        qk_sb[:, f * S + sc * 512 : f * S + (sc + 1) * 512],
                        qps[sc],
                    )

            def qk_f_steps(f, ps_a, scs=range(4)):
                """Single-sc ct-inner steps (~1.7us each) for filler use once
                the input DMAs have landed."""
                for sc in scs:
                    def step(f=f, sc=sc):
                        qps = ps_a.tile([128, 512], F32, tag="psw",
                                        name=f"qps{f}_{sc}")
                        for ct in range(NCT):
                            mm(
                                qps,
                                wqk_sb[:, f * 1024 + ct * 128 : f * 1024 + (ct + 1) * 128],
                                xt_sb[:, ct * S + sc * 512 : ct * S + (sc + 1) * 512],
                                start=(ct == 0),
                                stop=(ct == NCT - 1),
                            )
                        nc.vector.tensor_copy(
                            qk_sb[:, f * S + sc * 512 : f * S + (sc + 1) * 512],
                            qps,
                        )
                    yield step

            def v_steps(m0, m1, pool, tag):
                """V key-tiles m0..m1-1 -> vaug (strided, ones preserved)."""
                for m in range(m0, m1):
                    def step(m=m, pool=pool, tag=tag):
                        vps_full = pool.tile([128, 512], F32, tag=tag,
                                             name=f"vps{m}")
                        vps = vps_full[:, 0:256]
                        for ct in range(NCT):
                            mm(
                                vps,
                                xt_sb[:, ct * S + m * 128 : ct * S + (m + 1) * 128],
                                wv_sb[:, ct * 256 : (ct + 1) * 256],
                                start=(ct == 0),
                                stop=(ct == NCT - 1),
                            )
                        nc.vector.tensor_copy(
                            vaug[:, m * 260 : (m + 1) * 260].rearrange(
                                "p (h c) -> p h c", c=65
                            )[:, :, 0:64],
                            vps.rearrange("p (h c) -> p h c", c=64),
                        )
                    yield step

            def s_steps(pair, ch, pts):
                """Score+exp stream for block (pair, ch): 8 mg steps, each
                doing head A and head B over key tiles 2mg, 2mg+1."""
                qf, kf = pair, 2 + pair
                ptA = pt_pool.tile([128, MT * 512], BF16, tag="pt",
                                   name=f"ptA_{pair}_{ch}")
                ptB = pt_pool.tile([128, MT * 512], BF16, tag="pt",
                                   name=f"ptB_{pair}_{ch}")
                pts[(pair, ch)] = (ptA, ptB)
                for mg in range(MT // 2):
                    def step(pair=pair, ch=ch, mg=mg, ptA=ptA, ptB=ptB):
                        qf, kf = pair, 2 + pair
                        for hh, pt in ((0, ptA), (1, ptB)):
                            lo, hi = 64 * hh, 64 * hh + 64
                            sps = ps_s.tile([128, 1024], F32, tag="sps",
                                            name=f"s{pair}{ch}{mg}{hh}")
                            for j in range(2):
                                m = 2 * mg + j
                                mm(
                                    sps[:, j * 512 : (j + 1) * 512],
                                    qk_sb[lo:hi, kf * S + m * 128 : kf * S + (m + 1) * 128],
                                    qk_sb[lo:hi, qf * S + ch * 512 : qf * S + (ch + 1) * 512],
                                    start=True,
                                    stop=True,
                                )
                            nc.scalar.activation(
                                pt[:, 2 * mg * 512 : (2 * mg + 2) * 512],
                                sps,
                                EXP,
                            )
                    yield step

            def work_steps(pair, ch, pts, ps_w):
                """PV + normalize + transpose for block (pair, ch); when
                pair==1 also project+store token block qt of chunk ch."""
                ptA, ptB = pts[(pair, ch)]
                for qt in range(4):
                    def step(pair=pair, ch=ch, qt=qt, ptA=ptA, ptB=ptB):
                        osb = osb_pool.tile([128, 128], BF16, tag="osb",
                                            name=f"osb{pair}{ch}{qt}")
                        for hh, pt in ((0, ptA), (1, ptB)):
                            h = 2 * pair + hh
                            ops = ps_w.tile([128, 512], F32, tag="psw",
                                             name=f"o{pair}{ch}{qt}{hh}")
                            for m in range(MT):
                                mm(
                                    ops[:, 0:65],
                                    pt[:, m * 512 + qt * 128 : m * 512 + qt * 128 + 128],
                                    vaug[:, m * 260 + h * 65 : m * 260 + h * 65 + 65],
                                    start=(m == 0),
                                    stop=(m == MT - 1),
                                )
                            rz = rz_pool.tile([128, 1], F32, tag="rz",
                                              name=f"rz{pair}{ch}{qt}{hh}")
                            nc.vector.reciprocal(out=rz, in_=ops[:, 64:65])
                            nc.vector.tensor_scalar_mul(
                                osb[:, hh * 64 : hh * 64 + 64],
                                ops[:, 0:64],
                                rz,
                            )
                        # full [128 q, 128 d-pair] -> [128 d-pair, 128 q]
                        nc.sync.dma_start(
                            out=onorm[(pair, ch)][:, qt * 128 : (qt + 1) * 128],
                            in_=osb,
                            transpose=True,
                        )
                    yield step
                    if pair == 1:
                        def pstep(ch=ch, qt=qt):
                            st = ch * 4 + qt
                            ysb = ysb_pool.tile([128, C], F32, tag="ysb",
                                                name=f"ysb{ch}{qt}")
                            for fc in range(2):
                                yps = ps_w.tile([128, 512], F32, tag="psw",
                                                 name=f"y{ch}{qt}{fc}")
                                for p in range(2):
                                    mm(
                                        yps,
                                        onorm[(p, ch)][:, qt * 128 : (qt + 1) * 128],
                                        wp_sb[:, p * C + fc * 512 : p * C + (fc + 1) * 512],
                                        start=(p == 0),
                                        stop=(p == 1),
                                    )
                                nc.gpsimd.tensor_copy(
                                    ysb[:, fc * 512 : (fc + 1) * 512], yps
                                )
                            nc.sync.dma_start(
                                out=y_d[st * 128 : (st + 1) * 128, :], in_=ysb
                            )
                        yield pstep

            def chain(*gens):
                for g in gens:
                    yield from g

            def interleave(sgen, fgen):
                """Alternate one S step with one filler step; drain leftovers."""
                while True:
                    s = next(sgen, None)
                    if s is not None:
                        s()
                    f = next(fgen, None)
                    if f is not None:
                        f()
                    if s is None and f is None:
                        return

            def drain(g):
                for step in g:
                    step()

            # ---------------- pipelined schedule ----------------
            blocks = [(p, c) for p in range(2) for c in range(NCH)]
            pts = {}

            # during the input DMA stream: k-pair0 (4 sc ct-outer), then
            # the q columns S(p0,c0) needs
            qk_f_block(2, ps_w)
            drain(qk_f_steps(0, ps_w, scs=(0,)))
            # window 0: S(p0,c0) || remaining q0 + k1 columns
            interleave(
                s_steps(0, 0, pts),
                chain(
                    qk_f_steps(0, ps_w, scs=(1, 2, 3)),
                    qk_f_steps(3, ps_w),
                    qk_f_steps(1, ps_w, scs=(0,)),
                ),
            )
            # window 1: S(p0,c1) || q1 rest + V m0..4
            interleave(
                s_steps(0, 1, pts),
                chain(
                    qk_f_steps(1, ps_w, scs=(1, 2, 3)),
                    v_steps(0, 5, ps_w, 'psw'),
                ),
            )
            # window 2: S(p0,c2) || V m5..15 then work(B0)
            interleave(
                s_steps(0, 2, pts),
                chain(v_steps(5, MT, ps_w, 'psw'),
                      work_steps(*blocks[0], pts, ps_w)),
            )
            # windows 3..7: S(block k) || work(block k-2)
            for k in range(3, len(blocks)):
                interleave(
                    s_steps(*blocks[k], pts),
                    work_steps(*blocks[k - 2], pts, ps_w),
                )
            # tail
            for k in (len(blocks) - 2, len(blocks) - 1):
                drain(work_steps(*blocks[k], pts, ps_w))

    nc.compile()
    return nc


def make_core_inputs(x, Wqkv, Wproj):
    """Per-core input dicts. Core c: batch c//4, heads 4*(c%4) .. 4*(c%4)+3."""
    bf16 = mybir.dt.np(BF16)
    scale = D**-0.5  # 1/8, exact in bf16
    xts = [np.ascontiguousarray(x[b].T).astype(bf16) for b in range(B)]
    in_maps = []
    for core in range(8):
        b, hg = core // 4, core % 4
        heads = [HPC * hg + i for i in range(HPC)]
        rows_q = np.concatenate([Wqkv[D * h : D * (h + 1)] for h in heads]) * scale
        rows_k = np.concatenate([Wqkv[C + D * h : C + D * (h + 1)] for h in heads])
        wqk = np.ascontiguousarray(np.concatenate([rows_q, rows_k]).T).astype(bf16)
        wv = np.ascontiguousarray(
            np.concatenate([Wqkv[2 * C + D * h : 2 * C + D * (h + 1)] for h in heads]).T
        ).astype(bf16)
        wp = np.ascontiguousarray(
            np.concatenate([Wproj[:, D * h : D * (h + 1)] for h in heads], axis=1).T
        ).astype(bf16)
        in_maps.append({"xt": xts[b], "wqk": wqk, "wv": wv, "wp": wp})
    return in_maps


_EXEC_CACHE = {}


def _get_executor():
    """Build + jit the 8-core SPMD executable once per process."""
    if "fn" in _EXEC_CACHE:
        return _EXEC_CACHE
    import jax
    from jax.sharding import Mesh, PartitionSpec
    from jax.experimental.shard_map import shard_map
    from concourse import bass2jax
    from concourse.bass2jax import _bass_exec_p, partition_id_tensor

    nc = build_bass()
    bass2jax.install_neuronx_cc_hook()
    pid = nc.partition_id_tensor.name if nc.partition_id_tensor else None
    in_names, out_names, out_avals = [], [], []
    for alloc in nc.m.functions[0].allocations:
        if not isinstance(alloc, mybir.MemoryLocationSet):
            continue
        name = alloc.memorylocations[0].name
        if alloc.kind == "ExternalInput":
            if name != pid:
                in_names.append(name)
        elif alloc.kind == "ExternalOutput":
            out_names.append(name)
            out_avals.append(
                jax.core.ShapedArray(
                    tuple(alloc.tensor_shape), mybir.dt.np(alloc.dtype)
                )
            )
    n_params = len(in_names)
    all_names = list(in_names) + list(out_names) + ([pid] if pid else [])

    def body(*args):
        *ins, yb = args
        operands = list(ins) + [yb]
        if pid:
            operands.append(partition_id_tensor())
        outs = _bass_exec_p.bind(
            *operands,
            out_avals=tuple(out_avals),
            in_names=tuple(all_names),
            out_names=tuple(out_names),
            lowering_input_output_aliases=(),
            sim_require_finite=True,
            sim_require_nnan=True,
            nc=nc,
        )
        return outs[0]

    mesh = Mesh(np.asarray(jax.devices()[:8]), ("core",))
    fn = jax.jit(
        shard_map(
            body,
            mesh=mesh,
            in_specs=(PartitionSpec("core"),) * (n_params + 1),
            out_specs=PartitionSpec("core"),
            check_rep=False,
        ),
        donate_argnums=(n_params,),
    )
    _EXEC_CACHE.update(fn=fn, in_names=in_names)
    return _EXEC_CACHE


def kernel(x, Wqkv, Wproj, bproj):
    x = np.asarray(x, dtype=np.float32)
    Wqkv = np.asarray(Wqkv, dtype=np.float32)
    Wproj = np.asarray(Wproj, dtype=np.float32)
    bproj = np.asarray(bproj, dtype=np.float32)

    ex = _get_executor()
    in_maps = make_core_inputs(x, Wqkv, Wproj)
    glob_ins = [
        np.concatenate([np.asarray(m[name]) for m in in_maps], axis=0)
        for name in ex["in_names"]
    ]
    y0 = np.zeros((8 * S, C), np.float32)
    out = np.asarray(ex["fn"](*glob_ins, y0))  # [8*S, C]

    y = np.zeros((B, S, C), dtype=np.float32)
    for core in range(8):
        y[core // 4] += out[core * S : (core + 1) * S, :]
    y += bproj
    return y


# revision 13
# speedup vs baseline: 1.4516x; 1.0386x over previous
"""Multi-head attention block (B=2, N=2048, C=1024, H=16, D=64) on 8 TRN2 cores.

Sharding: core c -> batch b = c // 4, head-group hg = c % 4 (4 heads per core).
All matmuls in bf16 (1 cyc/row at any moving size in the cost model).
Per core:
  qkT  = Wqk_hg @ x_b^T          (f-tiles [2 heads x 64d, 2048 tok], bf16)
  V    = x_b @ Wv_hg^T           ([tok, 4h x 64d] natural layout -> vaug [V|1])
  S^T  = kT^T q-chunks           (K=64, two heads row-packed; [128 key, 512 q])
  P^T  = exp(S^T)                (ScalarE, no max-subtraction: scores ~ N(0,1))
  O|Z  = P^T-tile^T @ [V|1]      (q-major: out [128 q, 65]; V is the moving
                                  operand -> 65 rows/pass instead of 512)
  O_n  = O * (1/Z)               (DVE reciprocal + per-partition tensor_scalar)
  O^T  = DMA-transpose(O_n)      (XBAR, SBUF->SBUF, 2-byte)
  y    = O^T^T @ Wp^T            (K=128: two heads stacked per pass)
Host sums the 4 head-group partials per batch and adds bias.

Issue order is software-pipelined: S-matmul groups for block k+1 are
interleaved with PV/normalize/proj work of block k-2 so the ScalarE exp
stream (the bottleneck engine, ~133us) never starves.
"""

import numpy as np

import concourse.bass as bass
import concourse.tile as tile
from concourse import bacc, mybir

F32 = mybir.dt.float32
BF16 = mybir.dt.bfloat16
EXP = mybir.ActivationFunctionType.Exp

B, S, C = 2, 2048, 1024
H, D = 16, 64
HPC = 4            # heads per core
NCT = C // 128     # 8 contraction tiles
MT = S // 128      # 16 key tiles
NCH = S // 512     # 4 query chunks


def build_bass(loop_n=None):
    nc = bacc.Bacc("TRN2", target_bir_lowering=False)

    xt_d = nc.dram_tensor("xt", [C, S], BF16, kind="ExternalInput")
    wqk_d = nc.dram_tensor("wqk", [C, 512], BF16, kind="ExternalInput")
    wv_d = nc.dram_tensor("wv", [C, 256], BF16, kind="ExternalInput")
    wp_d = nc.dram_tensor("wp", [256, C], BF16, kind="ExternalInput")
    y_d = nc.dram_tensor("y", [S, C], F32, kind="ExternalOutput")

    def mm(out, lhsT, rhs, start, stop):
        nc.tensor.matmul(out, lhsT, rhs, start=start, stop=stop)

    with tile.TileContext(nc) as tc:
        with (
            tc.tile_pool(name="persist", bufs=1) as persist,
            tc.tile_pool(name="pt", bufs=4) as pt_pool,
            tc.tile_pool(name="rz", bufs=4) as rz_pool,
            tc.tile_pool(name="osb", bufs=2) as osb_pool,
            tc.tile_pool(name="ysb", bufs=2) as ysb_pool,
            tc.tile_pool(name="ps_s", bufs=2, space="PSUM") as ps_s,
            tc.tile_pool(name="ps_w", bufs=4, space="PSUM") as ps_w,
        ):
            # ---- persistent SBUF tiles ----
            # qkT f-tiles: 0=q_h0|q_h1, 1=q_h2|q_h3, 2=k_h0|k_h1, 3=k_h2|k_h3
            qk_sb = persist.tile([128, 4 * S], BF16, tag="qk")
            # V augmented per key tile m: [v_h0|1|v_h1|1|v_h2|1|v_h3|1]
            vaug = persist.tile([128, MT * 260], BF16, tag="vaug")
            wp_sb = persist.tile([128, 2 * C], BF16, tag="wp")
            xt_sb = persist.tile([128, NCT * S], BF16, tag="xt")
            wqk_sb = persist.tile([128, NCT * 512], BF16, tag="wqk")
            wv_sb = persist.tile([128, NCT * 256], BF16, tag="wv")
            # normalized-transposed O per (pair, ch): [128 = 2 heads x 64d, 512 q]
            onorm = [
                persist.tile([64, 0], BF16, tag="dummy")  # placeholder, replaced below
                for _ in range(0)
            ]
            onorm = {}
            for p in range(2):
                for c in range(NCH):
                    onorm[(p, c)] = persist.tile(
                        [128, 512], BF16, tag=f"onorm{p}{c}", name=f"onorm{p}{c}"
                    )

            nc.vector.memset(vaug, 1.0)

            def dma_wqk_f(f):
                # wqk_sb layout: [128, f*1024 + ct*128 + j]
                nc.sync.dma_start(
                    out=wqk_sb[:, f * 1024 : (f + 1) * 1024].rearrange(
                        "p (c j) -> p c j", c=NCT
                    ),
                    in_=wqk_d[:, f * 128 : (f + 1) * 128].rearrange(
                        "(c p) j -> p c j", p=128
                    ),
                )

            # DMA priority order: k-pair0 weights, then x, then the rest.
            dma_wqk_f(2)
            for ct in range(NCT):
                nc.sync.dma_start(
                    out=xt_sb[:, ct * S : (ct + 1) * S],
                    in_=xt_d[ct * 128 : (ct + 1) * 128, :],
                )
            for f in (0, 3, 1):
                dma_wqk_f(f)
            for ct in range(NCT):
                nc.sync.dma_start(
                    out=wv_sb[:, ct * 256 : (ct + 1) * 256],
                    in_=wv_d[ct * 128 : (ct + 1) * 128, :],
                )
            for p in range(2):
                nc.sync.dma_start(
                    out=wp_sb[:, p * C : (p + 1) * C],
                    in_=wp_d[p * 128 : (p + 1) * 128, :],
                )

            # ---------------- step generators ----------------

            def qk_f_block(f, ps_a, scs=range(4)):
                """qkT f-tile f, all scs ct-outer concurrently (needs
                len(scs) free ps_a bufs). Used during the input DMA stream."""
                qps = {
                    sc: ps_a.tile([128, 512], F32, tag="psw", name=f"qps{f}_{sc}")
                    for sc in scs
                }
                for ct in range(NCT):
                    for sc in scs:
                        mm(
                            qps[sc],
                            wqk_sb[:, f * 1024 + ct * 128 : f * 1024 + (ct + 1) * 128],
                            xt_sb[:, ct * S + sc * 512 : ct * S + (sc + 1) * 512],
                            start=(ct == 0),
                            stop=(ct == NCT - 1),
                        )
                for sc in scs:
                    nc.vector.tensor_copy(
                        qk_sb[:, f * S + sc * 512 : f * S + (sc + 1) * 512],
                        qps[sc],
                    )

            def qk_f_steps(f, ps_a, scs=range(4)):
                """Single-sc ct-inner steps (~1.7us each) for filler use once
                the input DMAs have landed."""
                for sc in scs:
                    def step(f=f, sc=sc):
                        qps = ps_a.tile([128, 512], F32, tag="psw",
                                        name=f"qps{f}_{sc}")
                        for ct in range(NCT):
                            mm(
                                qps,
                                wqk_sb[:, f * 1024 + ct * 128 : f * 1024 + (ct + 1) * 128],
                                xt_sb[:, ct * S + sc * 512 : ct * S + (sc + 1) * 512],
                                start=(ct == 0),
                                stop=(ct == NCT - 1),
                            )
                        nc.vector.tensor_copy(
                            qk_sb[:, f * S + sc * 512 : f * S + (sc + 1) * 512],
                            qps,
                        )
                    yield step

            def v_steps(m0, m1, pool, tag):
                """V key-tiles m0..m1-1 -> vaug (strided, ones preserved)."""
                for m in range(m0, m1):
                    def step(m=m, pool=pool, tag=tag):
                        vps_full = pool.tile([128, 512], F32, tag=tag,
                                             name=f"vps{m}")
                        vps = vps_full[:, 0:256]
                        for ct in range(NCT):
                            mm(
                                vps,
                                xt_sb[:, ct * S + m * 128 : ct * S + (m + 1) * 128],
                                wv_sb[:, ct * 256 : (ct + 1) * 256],
                                start=(ct == 0),
                                stop=(ct == NCT - 1),
                            )
                        nc.vector.tensor_copy(
                            vaug[:, m * 260 : (m + 1) * 260].rearrange(
                                "p (h c) -> p h c", c=65
                            )[:, :, 0:64],
                            vps.rearrange("p (h c) -> p h c", c=64),
                        )
                    yield step

            def s_steps(pair, ch, pts):
                """Score+exp stream for block (pair, ch): 8 mg steps, each
                doing head A and head B over key tiles 2mg, 2mg+1."""
                qf, kf = pair, 2 + pair
                ptA = pt_pool.tile([128, MT * 512], BF16, tag="pt",
                                   name=f"ptA_{pair}_{ch}")
                ptB = pt_pool.tile([128, MT * 512], BF16, tag="pt",
                                   name=f"ptB_{pair}_{ch}")
                pts[(pair, ch)] = (ptA, ptB)
                for mg in range(MT // 2):
                    def step(pair=pair, ch=ch, mg=mg, ptA=ptA, ptB=ptB):
                        qf, kf = pair, 2 + pair
                        for hh, pt in ((0, ptA), (1, ptB)):
                            lo, hi = 64 * hh, 64 * hh + 64
                            sps = ps_s.tile([128, 1024], F32, tag="sps",
                                            name=f"s{pair}{ch}{mg}{hh}")
                            for j in range(2):
                                m = 2 * mg + j
                                mm(
                                    sps[:, j * 512 : (j + 1) * 512],
                                    qk_sb[lo:hi, kf * S + m * 128 : kf * S + (m + 1) * 128],
                                    qk_sb[lo:hi, qf * S + ch * 512 : qf * S + (ch + 1) * 512],
                                    start=True,
                                    stop=True,
                                )
                            nc.scalar.activation(
                                pt[:, 2 * mg * 512 : (2 * mg + 2) * 512],
                                sps,
                                EXP,
                            )
                    yield step

            def work_steps(pair, ch, pts, ps_w):
                """PV + normalize + transpose for block (pair, ch); when
                pair==1 also project+store token block qt of chunk ch."""
                ptA, ptB = pts[(pair, ch)]
                for qt in range(4):
                    def step(pair=pair, ch=ch, qt=qt, ptA=ptA, ptB=ptB):
                        osb = osb_pool.tile([128, 128], BF16, tag="osb",
                                            name=f"osb{pair}{ch}{qt}")
                        for hh, pt in ((0, ptA), (1, ptB)):
                            h = 2 * pair + hh
                            ops = ps_w.tile([128, 512], F32, tag="psw",
                                             name=f"o{pair}{ch}{qt}{hh}")
                            for m in range(MT):
                                mm(
                                    ops[:, 0:65],
                                    pt[:, m * 512 + qt * 128 : m * 512 + qt * 128 + 128],
                                    vaug[:, m * 260 + h * 65 : m * 260 + h * 65 + 65],
                                    start=(m == 0),
                                    stop=(m == MT - 1),
                                )
                            rz = rz_pool.tile([128, 1], F32, tag="rz",
                                              name=f"rz{pair}{ch}{qt}{hh}")
                            nc.vector.reciprocal(out=rz, in_=ops[:, 64:65])
                            nc.vector.tensor_scalar_mul(
                                osb[:, hh * 64 : hh * 64 + 64],
                                ops[:, 0:64],
                                rz,
                            )
                        # full [128 q, 128 d-pair] -> [128 d-pair, 128 q]
                        nc.sync.dma_start(
                            out=onorm[(pair, ch)][:, qt * 128 : (qt + 1) * 128],
                            in_=osb,
                            transpose=True,
                        )
                    yield step
                    if pair == 1:
                        def pstep(ch=ch, qt=qt):
                            st = ch * 4 + qt
                            ysb = ysb_pool.tile([128, C], F32, tag="ysb",
                                                name=f"ysb{ch}{qt}")
                            for fc in range(2):
                                yps = ps_w.tile([128, 512], F32, tag="psw",
                                                 name=f"y{ch}{qt}{fc}")
                                for p in range(2):
                                    mm(
                                        yps,
                                        onorm[(p, ch)][:, qt * 128 : (qt + 1) * 128],
                                        wp_sb[:, p * C + fc * 512 : p * C + (fc + 1) * 512],
                                        start=(p == 0),
                                        stop=(p == 1),
                                    )
                                nc.gpsimd.tensor_copy(
                                    ysb[:, fc * 512 : (fc + 1) * 512], yps
                                )
                            nc.sync.dma_start(
                                out=y_d[st * 128 : (st + 1) * 128, :], in_=ysb
                            )
                        yield pstep

            def chain(*gens):
                for g in gens:
                    yield from g

            def interleave(sgen, fgen):
                """Alternate one S step with one filler step; drain leftovers."""
                while True:
                    s = next(sgen, None)
                    if s is not None:
                        s()
                    f = next(fgen, None)
                    if f is not None:
                        f()
                    if s is None and f is None:
                        return

            def drain(g):
                for step in g:
                    step()

            # ---------------- pipelined schedule ----------------
            blocks = [(p, c) for p in range(2) for c in range(NCH)]
            pts = {}

            # during the input DMA stream: k-pair0 (4 sc ct-outer), then
            # the q columns S(p0,c0) needs
            qk_f_block(2, ps_w)
            drain(qk_f_steps(0, ps_w, scs=(0,)))
            # window 0: S(p0,c0) || q0 rest + V m0..4
            interleave(
                s_steps(0, 0, pts),
                chain(
                    qk_f_steps(0, ps_w, scs=(1, 2, 3)),
                    v_steps(0, 5, ps_w, 'psw'),
                ),
            )
            # window 1: S(p0,c1) || V m5..15
            interleave(
                s_steps(0, 1, pts),
                v_steps(5, MT, ps_w, 'psw'),
            )
            # window 2: S(p0,c2) || k1 weights + work(B0)
            interleave(
                s_steps(0, 2, pts),
                chain(qk_f_steps(3, ps_w),
                      work_steps(*blocks[0], pts, ps_w)),
            )
            # window 3: S(p0,c3) || q1 weights + work(B1)
            interleave(
                s_steps(0, 3, pts),
                chain(qk_f_steps(1, ps_w),
                      work_steps(*blocks[1], pts, ps_w)),
            )
            # windows 4..7: S(block k) || work(block k-2)
            for k in range(4, len(blocks)):
                interleave(
                    s_steps(*blocks[k], pts),
                    work_steps(*blocks[k - 2], pts, ps_w),
                )
            # tail
            for k in (len(blocks) - 2, len(blocks) - 1):
                drain(work_steps(*blocks[k], pts, ps_w))

    nc.compile()
    return nc


def make_core_inputs(x, Wqkv, Wproj):
    """Per-core input dicts. Core c: batch c//4, heads 4*(c%4) .. 4*(c%4)+3."""
    bf16 = mybir.dt.np(BF16)
    scale = D**-0.5  # 1/8, exact in bf16
    xts = [np.ascontiguousarray(x[b].T).astype(bf16) for b in range(B)]
    in_maps = []
    for core in range(8):
        b, hg = core // 4, core % 4
        heads = [HPC * hg + i for i in range(HPC)]
        rows_q = np.concatenate([Wqkv[D * h : D * (h + 1)] for h in heads]) * scale
        rows_k = np.concatenate([Wqkv[C + D * h : C + D * (h + 1)] for h in heads])
        wqk = np.ascontiguousarray(np.concatenate([rows_q, rows_k]).T).astype(bf16)
        wv = np.ascontiguousarray(
            np.concatenate([Wqkv[2 * C + D * h : 2 * C + D * (h + 1)] for h in heads]).T
        ).astype(bf16)
        wp = np.ascontiguousarray(
            np.concatenate([Wproj[:, D * h : D * (h + 1)] for h in heads], axis=1).T
        ).astype(bf16)
        in_maps.append({"xt": xts[b], "wqk": wqk, "wv": wv, "wp": wp})
    return in_maps


_EXEC_CACHE = {}


def _get_executor():
    """Build + jit the 8-core SPMD executable once per process."""
    if "fn" in _EXEC_CACHE:
        return _EXEC_CACHE
    import jax
    from jax.sharding import Mesh, PartitionSpec
    from jax.experimental.shard_map import shard_map
    from concourse import bass2jax
    from concourse.bass2jax import _bass_exec_p, partition_id_tensor

    nc = build_bass()
    bass2jax.install_neuronx_cc_hook()
    pid = nc.partition_id_tensor.name if nc.partition_id_tensor else None
    in_names, out_names, out_avals = [], [], []
    for alloc in nc.m.functions[0].allocations:
        if not isinstance(alloc, mybir.MemoryLocationSet):
            continue
        name = alloc.memorylocations[0].name
        if alloc.kind == "ExternalInput":
            if name != pid:
                in_names.append(name)
        elif alloc.kind == "ExternalOutput":
            out_names.append(name)
            out_avals.append(
                jax.core.ShapedArray(
                    tuple(alloc.tensor_shape), mybir.dt.np(alloc.dtype)
                )
            )
    n_params = len(in_names)
    all_names = list(in_names) + list(out_names) + ([pid] if pid else [])

    def body(*args):
        *ins, yb = args
        operands = list(ins) + [yb]
        if pid:
            operands.append(partition_id_tensor())
        outs = _bass_exec_p.bind(
            *operands,
            out_avals=tuple(out_avals),
            in_names=tuple(all_names),
            out_names=tuple(out_names),
            lowering_input_output_aliases=(),
            sim_require_finite=True,
            sim_require_nnan=True,
            nc=nc,
        )
        return outs[0]

    mesh = Mesh(np.asarray(jax.devices()[:8]), ("core",))
    fn = jax.jit(
        shard_map(
            body,
            mesh=mesh,
            in_specs=(PartitionSpec("core"),) * (n_params + 1),
            out_specs=PartitionSpec("core"),
            check_rep=False,
        ),
        donate_argnums=(n_params,),
    )
    _EXEC_CACHE.update(fn=fn, in_names=in_names)
    return _EXEC_CACHE


def kernel(x, Wqkv, Wproj, bproj):
    x = np.asarray(x, dtype=np.float32)
    Wqkv = np.asarray(Wqkv, dtype=np.float32)
    Wproj = np.asarray(Wproj, dtype=np.float32)
    bproj = np.asarray(bproj, dtype=np.float32)

    ex = _get_executor()
    in_maps = make_core_inputs(x, Wqkv, Wproj)
    glob_ins = [
        np.concatenate([np.asarray(m[name]) for m in in_maps], axis=0)
        for name in ex["in_names"]
    ]
    y0 = np.zeros((8 * S, C), np.float32)
    out = np.asarray(ex["fn"](*glob_ins, y0))  # [8*S, C]

    y = np.zeros((B, S, C), dtype=np.float32)
    for core in range(8):
        y[core // 4] += out[core * S : (core + 1) * S, :]
    y += bproj
    return y
